# revision 22
# baseline (speedup 1.0000x reference)
"""Causal multi-head attention on 8 TRN2 NeuronCores.

Reference (per batch b):
    q,k,v = x @ W^T  (W: [d_out, d_in]), split into H=16 heads of dk=64
    attn  = softmax(causal(q k^T / sqrt(dk))) v
    y     = concat_heads(attn) @ W_o^T

Sharding (8 cores): core c -> batch b = c//4, head group g = c%4 (4 heads,
256 channels).  w_q/w_k/w_v column-sharded by head; w_o row-sharded — each
core computes a partial y[b] over its 256 channels; the host sums the 4
partials per batch (the unshard step).

Device kernel layout choices:
  - All matmul inputs bf16 (host-cast), f32 PSUM accumulation.
  - x is staged transposed (x^T: [d, s]) so q^T/k^T ([e_local, s]) come
    straight out of the PE and serve as lhsT/rhs of the scores matmul.
  - Scores are computed transposed: S^T[kpos, q], contraction d=64 per
    head, two heads packed in the 128-row PE array via row tiling
    (tile_position (0,0)/(64,0)) writing one 2-bank PSUM pair, so a single
    ScalarE ACTIVATE does exp for both heads (amortizes the 352-cycle
    ACTIVATE overhead).
  - Softmax without max-subtraction (scores are O(10), exp is safe in f32),
    exp output bf16.
  - Causality at tile granularity: kpos-tile i of q-chunk j is skipped when
    fully masked; multiplied by a precomputed 0/1 mask on the 4 diagonal
    tiles (mask depends only on r = i - 4j).
  - P @ V via V_aug = [V | 1]: lhsT = V_aug [kpos, 65]; row 64 of the
    accumulated [65, q] output is the softmax denominator.
  - attn^T = out[0:64] * (1/denominator broadcast), using the fast DVE
    reciprocal (18-bit) — the exact one runs at 1/8 rate and costs 3.3us
    per row.
  - y = attn^T.T @ w_o^T slices, f32 out.  The y-projection matmuls for
    q-chunk j are emitted right after chunk j's attention so the scheduler
    can fill PE idle time during the ACT-bound attention phase (keeps the
    HAM clock-gate warm).
"""

import numpy as np
import ml_dtypes

B = 2
S = 2048
D = 1024
H = 16
DK = 64
NCORES = 8
EL = 256  # local channels per core (4 heads)
QW = 512  # q-chunk width (free dim of scores matmuls)
NJ = S // QW  # 4 q-chunks
NKT = S // 128  # 16 kpos tiles

_CACHE = {}


def _build():
    import concourse.bass as bass
    import concourse.mybir as mybir
    import concourse.tile as tile
    from concourse import bacc

    f32 = mybir.dt.float32
    bf16 = mybir.dt.bfloat16
    ts = bass.ts
    Exp = mybir.ActivationFunctionType.Exp

    nc = bacc.Bacc("TRN2", num_devices=NCORES)
    xT_d = nc.dram_tensor("xT", [D, S], bf16, kind="ExternalInput")
    wqT_d = nc.dram_tensor("wqT", [D, EL], bf16, kind="ExternalInput")
    wkT_d = nc.dram_tensor("wkT", [D, EL], bf16, kind="ExternalInput")
    wvT_d = nc.dram_tensor("wvT", [D, EL], bf16, kind="ExternalInput")
    woT_d = nc.dram_tensor("woT", [EL, D], bf16, kind="ExternalInput")
    y_d = nc.dram_tensor("y", [S, D], f32, kind="ExternalOutput")

    DT = D // 128  # 8 d-tiles
    ST = S // 128  # 16 s-tiles

    with tile.TileContext(nc) as tc:
        with (
            tc.tile_pool(name="big", bufs=1) as big,
            tc.tile_pool(name="work", bufs=3) as work,
            tc.tile_pool(name="psum", bufs=2, space="PSUM") as psum,
        ):
            xT = big.tile([128, DT, S], bf16)  # x^T (d, s)
            wqT = big.tile([128, DT, EL], bf16)
            wkT = big.tile([128, DT, EL], bf16)
            wvT = big.tile([128, DT, EL], bf16)
            woT = big.tile([128, EL // 128, D], bf16)
            qT = big.tile([128, 2, S], bf16)  # (e_local, s)
            kT = big.tile([128, 2, S], bf16)
            vA = big.tile([128, ST, 4, DK + 1], bf16)  # (s%128, s//128, h, dv+1)
            aT = big.tile([128, 2, S], bf16)  # attn^T (d_local, s)
            masks = big.tile([128, 4, QW], bf16)

            # ---- input DMAs (weights first so stage B can start early) ----
            nc.sync.dma_start(
                wqT[:], wqT_d.ap().rearrange("(ko p) e -> p ko e", p=128)
            )
            nc.sync.dma_start(
                wkT[:], wkT_d.ap().rearrange("(ko p) e -> p ko e", p=128)
            )
            xT_r = xT_d.ap().rearrange("(ko p) s -> p ko s", p=128)
            for kd in range(DT):
                nc.sync.dma_start(xT[:, kd, :], xT_r[:, kd, :])
            nc.sync.dma_start(
                wvT[:], wvT_d.ap().rearrange("(ko p) e -> p ko e", p=128)
            )
            nc.sync.dma_start(
                woT[:], woT_d.ap().rearrange("(ko p) e -> p ko e", p=128)
            )

            # ---- constants: diagonal masks + ones column of V_aug ----
            # mask[r][p, f] = 1.0 iff f >= p + 128*r  (kpos <= q within tile)
            nc.gpsimd.memset(masks[:], 1.0)
            for r in range(4):
                nc.gpsimd.affine_select(
                    out=masks[:, r, :],
                    in_=masks[:, r, :],
                    compare_op=mybir.AluOpType.is_ge,
                    fill=0.0,
                    base=-128 * r,
                    pattern=[[1, QW]],
                    channel_multiplier=-1,
                )
            nc.gpsimd.memset(vA[:, :, :, DK], 1.0)

            # ---- stage B: q^T, k^T = (W^T)^T-slices.T @ x^T ; v natural ----
            for w_sb, outT in ((wqT, qT), (wkT, kT)):
                for et in range(2):
                    for sj in range(NJ):
                        ps = psum.tile([128, QW], f32, tag="sc")
                        for kd in range(DT):
                            nc.tensor.matmul(
                                ps[:],
                                w_sb[:, kd, ts(et, 128)],
                                xT[:, kd, ts(sj, QW)],
                                start=(kd == 0),
                                stop=(kd == DT - 1),
                            )
                        nc.vector.tensor_copy(outT[:, et, ts(sj, QW)], ps[:])
            for st in range(ST):
                ps = psum.tile([128, EL], f32, tag="sc")
                for kd in range(DT):
                    nc.tensor.matmul(
                        ps[:],
                        xT[:, kd, ts(st, 128)],
                        wvT[:, kd, :],
                        start=(kd == 0),
                        stop=(kd == DT - 1),
                    )
                nc.vector.tensor_copy(
                    vA[:, st, :, 0:DK],
                    ps[:].rearrange("p (h e) -> p h e", h=4),
                )

            # ---- stage C + software-pipelined stage D ----
            # stage D for chunk j is emitted during chunk j+1's attention so
            # the PE has independent fill-work while chunk j+1's divisions
            # drain (avoids >3.4us PE gaps that re-throttle the HAM clock).
            def stage_d(jd):
                for st in range(4 * jd, 4 * jd + 4):
                    for eo in range(2):
                        ps = psum.tile([128, QW], f32, tag="sc")
                        for kd in range(EL // 128):
                            nc.tensor.matmul(
                                ps[:],
                                aT[:, kd, ts(st, 128)],
                                woT[:, kd, ts(eo, QW)],
                                start=(kd == 0),
                                stop=(kd == EL // 128 - 1),
                            )
                        yt = work.tile([128, QW], f32, tag="yout", bufs=3)
                        nc.vector.tensor_copy(yt[:], ps[:])
                        nc.sync.dma_start(
                            y_d.ap().rearrange("(so p) e -> p so e", p=128)[
                                :, st, ts(eo, QW)
                            ],
                            yt[:],
                        )

            for j in range(NJ):
                ilast = 4 * j + 3
                if j > 0:
                    stage_d(j - 1)
                for hp in range(2):  # head pair = e-tile of qT/kT
                    oa0 = psum.tile([128, QW], f32, tag="oa", bufs=4)
                    oa1 = psum.tile([128, QW], f32, tag="oa", bufs=4)
                    for i in range(ilast + 1):
                        sc = psum.tile([128, 2 * QW], f32, tag="sc")
                        nc.tensor.matmul(
                            sc[:, 0:QW],
                            kT[0:64, hp, ts(i, 128)],
                            qT[0:64, hp, ts(j, QW)],
                            tile_position=(0, 0),
                        )
                        nc.tensor.matmul(
                            sc[:, QW : 2 * QW],
                            kT[64:128, hp, ts(i, 128)],
                            qT[64:128, hp, ts(j, QW)],
                            tile_position=(64, 0),
                        )
                        e01 = work.tile([128, 2 * QW], bf16, tag="exps", bufs=4)
                        nc.scalar.activation(e01[:], sc[:], Exp, scale=0.125)
                        if i >= 4 * j:  # diagonal tile: causal mask
                            # only cols f < (r+1)*128 can be masked (f >= p + 128r)
                            r = i - 4 * j
                            mw = (r + 1) * 128
                            nc.vector.tensor_mul(
                                e01[:, 0:mw], e01[:, 0:mw], masks[:, r, 0:mw]
                            )
                            nc.vector.tensor_mul(
                                e01[:, QW : QW + mw],
                                e01[:, QW : QW + mw],
                                masks[:, r, 0:mw],
                            )
                        nc.tensor.matmul(
                            oa0[0 : DK + 1, :],
                            vA[:, i, 2 * hp, :],
                            e01[:, 0:QW],
                            start=(i == 0),
                            stop=(i == ilast),
                        )
                        nc.tensor.matmul(
                            oa1[0 : DK + 1, :],
                            vA[:, i, 2 * hp + 1, :],
                            e01[:, QW : 2 * QW],
                            start=(i == 0),
                            stop=(i == ilast),
                        )
                    for hh, oa in ((0, oa0), (1, oa1)):
                        h = 2 * hp + hh
                        # copy PV accumulator (+denominator row) to SBUF so the
                        # PSUM bank frees immediately; division runs off the
                        # critical path.
                        # Free the PSUM bank quickly: denominator row -> ua[0],
                        # numerators -> ua[32:96] (engine APs need 32-aligned
                        # partition bases; custom ops need base 0 inputs).
                        un = work.tile([64, QW], f32, tag="un", bufs=4)
                        nc.vector.tensor_copy(un[:], oa[0:DK, :])
                        dn = work.tile([1, QW], f32, tag="dn", bufs=2)
                        nc.vector.tensor_copy(dn[:], oa[DK : DK + 1, :])
                        rc = work.tile([1, QW], f32, tag="rc", bufs=2)
                        nc.vector.reciprocal_approx_fast(out=rc[:], in_=dn[:])
                        bc = work.tile([64, QW], f32, tag="bc", bufs=2)
                        nc.gpsimd.partition_broadcast(bc[:], rc[:])
                        nc.vector.tensor_mul(
                            aT[(h % 2) * 64 : (h % 2) * 64 + 64, h // 2, ts(j, QW)],
                            un[:],
                            bc[:],
                        )

            stage_d(NJ - 1)

    nc.compile()
    return nc


def _get_nc():
    if "nc" not in _CACHE:
        _CACHE["nc"] = _build()
    return _CACHE["nc"]


def kernel(x, w_q, w_k, w_v, w_o, _trace=False, _trace_cores=None):
    from concourse.bass_utils import run_bass_kernel_spmd

    nc = _get_nc()
    bf = ml_dtypes.bfloat16
    in_maps = []
    for c in range(NCORES):
        b = c // 4
        g = c % 4
        ch = slice(g * EL, (g + 1) * EL)
        in_maps.append(
            {
                "xT": np.ascontiguousarray(x[b].T).astype(bf),
                "wqT": np.ascontiguousarray(w_q[ch, :].T).astype(bf),
                "wkT": np.ascontiguousarray(w_k[ch, :].T).astype(bf),
                "wvT": np.ascontiguousarray(w_v[ch, :].T).astype(bf),
                "woT": np.ascontiguousarray(w_o[:, ch].T).astype(bf),
            }
        )
    res = run_bass_kernel_spmd(
        nc,
        in_maps,
        core_ids=list(range(NCORES)),
        trace=_trace,
        trace_cores=_trace_cores,
    )
    _CACHE["last_results"] = res
    y = np.zeros((B, S, D), np.float32)
    for c in range(NCORES):
        y[c // 4] += res.results[c]["y"]
    return y


# revision 24
# speedup vs baseline: 1.1063x; 1.1063x over previous
"""Causal multi-head attention on 8 TRN2 NeuronCores.

Reference (per batch b):
    q,k,v = x @ W^T  (W: [d_out, d_in]), split into H=16 heads of dk=64
    attn  = softmax(causal(q k^T / sqrt(dk))) v
    y     = concat_heads(attn) @ W_o^T

Sharding (8 cores): core c -> batch b = c//4, head group g = c%4 (4 heads,
256 channels).  w_q/w_k/w_v column-sharded by head; w_o row-sharded — each
core computes a partial y[b] over its 256 channels; the host sums the 4
partials per batch (the unshard step).

Device kernel layout choices:
  - All matmul inputs bf16 (host-cast), f32 PSUM accumulation.
  - x is staged transposed (x^T: [d, s]) so q^T/k^T ([e_local, s]) come
    straight out of the PE and serve as lhsT/rhs of the scores matmul.
  - Scores are computed transposed: S^T[kpos, q], contraction d=64 per
    head, two heads packed in the 128-row PE array via row tiling
    (tile_position (0,0)/(64,0)) writing one 2-bank PSUM pair, so a single
    ScalarE ACTIVATE does exp for both heads (amortizes the 352-cycle
    ACTIVATE overhead).
  - Softmax without max-subtraction (scores are O(10), exp is safe in f32),
    exp output bf16.
  - Causality at tile granularity: kpos-tile i of q-chunk j is skipped when
    fully masked; multiplied by a precomputed 0/1 mask on the 4 diagonal
    tiles (mask depends only on r = i - 4j).
  - P @ V via V_aug = [V | 1]: lhsT = V_aug [kpos, 65]; row 64 of the
    accumulated [65, q] output is the softmax denominator.
  - attn^T = out[0:64] * (1/denominator broadcast), using the fast DVE
    reciprocal (18-bit) — the exact one runs at 1/8 rate and costs 3.3us
    per row.
  - y = attn^T.T @ w_o^T slices, f32 out.  The y-projection matmuls for
    q-chunk j are emitted right after chunk j's attention so the scheduler
    can fill PE idle time during the ACT-bound attention phase (keeps the
    HAM clock-gate warm).
"""

import numpy as np
import ml_dtypes

B = 2
S = 2048
D = 1024
H = 16
DK = 64
NCORES = 8
EL = 256  # local channels per core (4 heads)
QW = 512  # q-chunk width (free dim of scores matmuls)
NJ = S // QW  # 4 q-chunks
NKT = S // 128  # 16 kpos tiles

_CACHE = {}


def _build():
    import concourse.bass as bass
    import concourse.mybir as mybir
    import concourse.tile as tile
    from concourse import bacc

    f32 = mybir.dt.float32
    bf16 = mybir.dt.bfloat16
    ts = bass.ts
    Exp = mybir.ActivationFunctionType.Exp

    nc = bacc.Bacc("TRN2", num_devices=NCORES)
    xT_d = nc.dram_tensor("xT", [D, S], bf16, kind="ExternalInput")
    wqT_d = nc.dram_tensor("wqT", [D, EL], bf16, kind="ExternalInput")
    wkT_d = nc.dram_tensor("wkT", [D, EL], bf16, kind="ExternalInput")
    wvT_d = nc.dram_tensor("wvT", [D, EL], bf16, kind="ExternalInput")
    woT_d = nc.dram_tensor("woT", [EL, D], bf16, kind="ExternalInput")
    y_d = nc.dram_tensor("y", [S, D], f32, kind="ExternalOutput")

    DT = D // 128  # 8 d-tiles
    ST = S // 128  # 16 s-tiles

    with tile.TileContext(nc) as tc:
        with (
            tc.tile_pool(name="big", bufs=1) as big,
            tc.tile_pool(name="work", bufs=3) as work,
            tc.tile_pool(name="psum", bufs=2, space="PSUM") as psum,
        ):
            xT = big.tile([128, DT, S], bf16)  # x^T (d, s)
            wqT = big.tile([128, DT, EL], bf16)
            wkT = big.tile([128, DT, EL], bf16)
            wvT = big.tile([128, DT, EL], bf16)
            woT = big.tile([128, EL // 128, D], bf16)
            qT = big.tile([128, 2, S], bf16)  # (e_local, s)
            kT = big.tile([128, 2, S], bf16)
            vA = big.tile([128, ST, 4, DK + 1], bf16)  # (s%128, s//128, h, dv+1)
            aT = big.tile([128, 2, S], bf16)  # attn^T (d_local, s)
            masks = big.tile([128, 4, QW], bf16)

            # ---- input DMAs (weights first so stage B can start early) ----
            nc.sync.dma_start(
                wqT[:], wqT_d.ap().rearrange("(ko p) e -> p ko e", p=128)
            )
            nc.sync.dma_start(
                wkT[:], wkT_d.ap().rearrange("(ko p) e -> p ko e", p=128)
            )
            xT_r = xT_d.ap().rearrange("(ko p) s -> p ko s", p=128)
            for kd in range(DT):
                nc.sync.dma_start(xT[:, kd, :], xT_r[:, kd, :])
            nc.sync.dma_start(
                wvT[:], wvT_d.ap().rearrange("(ko p) e -> p ko e", p=128)
            )
            nc.sync.dma_start(
                woT[:], woT_d.ap().rearrange("(ko p) e -> p ko e", p=128)
            )

            # ---- constants: diagonal masks + ones column of V_aug ----
            # mask[r][p, f] = 1.0 iff f >= p + 128*r  (kpos <= q within tile)
            nc.gpsimd.memset(masks[:], 1.0)
            for r in range(4):
                nc.gpsimd.affine_select(
                    out=masks[:, r, :],
                    in_=masks[:, r, :],
                    compare_op=mybir.AluOpType.is_ge,
                    fill=0.0,
                    base=-128 * r,
                    pattern=[[1, QW]],
                    channel_multiplier=-1,
                )
            nc.gpsimd.memset(vA[:, :, :, DK], 1.0)

            # ---- stage B: q^T, k^T = (W^T)^T-slices.T @ x^T ; v natural ----
            for w_sb, outT in ((wqT, qT), (wkT, kT)):
                for et in range(2):
                    for sj in range(NJ):
                        ps = psum.tile([128, QW], f32, tag="sc")
                        for kd in range(DT):
                            nc.tensor.matmul(
                                ps[:],
                                w_sb[:, kd, ts(et, 128)],
                                xT[:, kd, ts(sj, QW)],
                                start=(kd == 0),
                                stop=(kd == DT - 1),
                            )
                        nc.vector.tensor_copy(outT[:, et, ts(sj, QW)], ps[:])
            for st in range(ST):
                ps = psum.tile([128, EL], f32, tag="sc")
                for kd in range(DT):
                    nc.tensor.matmul(
                        ps[:],
                        xT[:, kd, ts(st, 128)],
                        wvT[:, kd, :],
                        start=(kd == 0),
                        stop=(kd == DT - 1),
                    )
                nc.vector.tensor_copy(
                    vA[:, st, :, 0:DK],
                    ps[:].rearrange("p (h e) -> p h e", h=4),
                )

            # ---- stage C + software-pipelined stage D ----
            # stage D for chunk j is emitted during chunk j+1's attention so
            # the PE has independent fill-work while chunk j+1's divisions
            # drain (avoids >3.4us PE gaps that re-throttle the HAM clock).
            def stage_d(jd):
                for st in range(4 * jd, 4 * jd + 4):
                    for eo in range(2):
                        ps = psum.tile([128, QW], f32, tag="sc")
                        for kd in range(EL // 128):
                            nc.tensor.matmul(
                                ps[:],
                                aT[:, kd, ts(st, 128)],
                                woT[:, kd, ts(eo, QW)],
                                start=(kd == 0),
                                stop=(kd == EL // 128 - 1),
                            )
                        yt = work.tile([128, QW], f32, tag="yout", bufs=3)
                        nc.vector.tensor_copy(yt[:], ps[:])
                        nc.sync.dma_start(
                            y_d.ap().rearrange("(so p) e -> p so e", p=128)[
                                :, st, ts(eo, QW)
                            ],
                            yt[:],
                        )

            for j in range(NJ):
                ilast = 4 * j + 3
                for hp in range(2):  # head pair = e-tile of qT/kT
                    if j > 0 and hp == 1:
                        # chunk j-1's output projection: emitted mid-chunk so
                        # its aT inputs are long-ready and the PE never waits
                        # on them at a chunk boundary.
                        stage_d(j - 1)
                    oa0 = psum.tile([128, QW], f32, tag="oa", bufs=4)
                    oa1 = psum.tile([128, QW], f32, tag="oa", bufs=4)
                    for i in range(ilast + 1):
                        sc = psum.tile([128, 2 * QW], f32, tag="sc")
                        nc.tensor.matmul(
                            sc[:, 0:QW],
                            kT[0:64, hp, ts(i, 128)],
                            qT[0:64, hp, ts(j, QW)],
                            tile_position=(0, 0),
                        )
                        nc.tensor.matmul(
                            sc[:, QW : 2 * QW],
                            kT[64:128, hp, ts(i, 128)],
                            qT[64:128, hp, ts(j, QW)],
                            tile_position=(64, 0),
                        )
                        e01 = work.tile([128, 2 * QW], bf16, tag="exps", bufs=4)
                        nc.scalar.activation(e01[:], sc[:], Exp, scale=0.125)
                        if i >= 4 * j:  # diagonal tile: causal mask
                            # only cols f < (r+1)*128 can be masked (f >= p + 128r)
                            r = i - 4 * j
                            mw = (r + 1) * 128
                            nc.vector.tensor_mul(
                                e01[:, 0:mw], e01[:, 0:mw], masks[:, r, 0:mw]
                            )
                            nc.vector.tensor_mul(
                                e01[:, QW : QW + mw],
                                e01[:, QW : QW + mw],
                                masks[:, r, 0:mw],
                            )
                        nc.tensor.matmul(
                            oa0[0 : DK + 1, :],
                            vA[:, i, 2 * hp, :],
                            e01[:, 0:QW],
                            start=(i == 0),
                            stop=(i == ilast),
                        )
                        nc.tensor.matmul(
                            oa1[0 : DK + 1, :],
                            vA[:, i, 2 * hp + 1, :],
                            e01[:, QW : 2 * QW],
                            start=(i == 0),
                            stop=(i == ilast),
                        )
                    for hh, oa in ((0, oa0), (1, oa1)):
                        h = 2 * hp + hh
                        # copy PV accumulator (+denominator row) to SBUF so the
                        # PSUM bank frees immediately; division runs off the
                        # critical path.
                        # Free the PSUM bank quickly: denominator row -> ua[0],
                        # numerators -> ua[32:96] (engine APs need 32-aligned
                        # partition bases; custom ops need base 0 inputs).
                        dn = work.tile([1, QW], f32, tag="dn", bufs=2)
                        nc.vector.tensor_copy(dn[:], oa[DK : DK + 1, :])
                        rc = work.tile([1, QW], f32, tag="rc", bufs=2)
                        nc.vector.reciprocal_approx_fast(out=rc[:], in_=dn[:])
                        bc = work.tile([64, QW], f32, tag="bc", bufs=2)
                        nc.gpsimd.partition_broadcast(bc[:], rc[:])
                        nc.vector.tensor_mul(
                            aT[(h % 2) * 64 : (h % 2) * 64 + 64, h // 2, ts(j, QW)],
                            oa[0:DK, :],
                            bc[:],
                        )

            stage_d(NJ - 1)

    nc.compile()
    return nc


def _get_nc():
    if "nc" not in _CACHE:
        _CACHE["nc"] = _build()
    return _CACHE["nc"]


def kernel(x, w_q, w_k, w_v, w_o, _trace=False, _trace_cores=None):
    from concourse.bass_utils import run_bass_kernel_spmd

    nc = _get_nc()
    bf = ml_dtypes.bfloat16
    in_maps = []
    for c in range(NCORES):
        b = c // 4
        g = c % 4
        ch = slice(g * EL, (g + 1) * EL)
        in_maps.append(
            {
                "xT": np.ascontiguousarray(x[b].T).astype(bf),
                "wqT": np.ascontiguousarray(w_q[ch, :].T).astype(bf),
                "wkT": np.ascontiguousarray(w_k[ch, :].T).astype(bf),
                "wvT": np.ascontiguousarray(w_v[ch, :].T).astype(bf),
                "woT": np.ascontiguousarray(w_o[:, ch].T).astype(bf),
            }
        )
    res = run_bass_kernel_spmd(
        nc,
        in_maps,
        core_ids=list(range(NCORES)),
        trace=_trace,
        trace_cores=_trace_cores,
    )
    _CACHE["last_results"] = res
    y = np.zeros((B, S, D), np.float32)
    for c in range(NCORES):
        y[c // 4] += res.results[c]["y"]
    return y


# revision 26
# speedup vs baseline: 1.1709x; 1.0583x over previous
"""Causal multi-head attention on 8 TRN2 NeuronCores.

Reference (per batch b):
    q,k,v = x @ W^T  (W: [d_out, d_in]), split into H=16 heads of dk=64
    attn  = softmax(causal(q k^T / sqrt(dk))) v
    y     = concat_heads(attn) @ W_o^T

Sharding (8 cores): core c -> batch b = c//4, head group g = c%4 (4 heads,
256 channels).  w_q/w_k/w_v column-sharded by head; w_o row-sharded — each
core computes a partial y[b] over its 256 channels; the host sums the 4
partials per batch (the unshard step).

Device kernel structure (one software pipeline over s-chunks of 512):
    for sj: project q/k/v for s-chunk sj (PE-dense), then attention for
    q-chunk sj (ScalarE-exp-dense), with the y-projection of chunk sj-1
    folded into the middle.  Interleaving the PE-heavy projection work with
    the ACT-heavy attention work keeps both engines streaming and avoids
    >3.4us PE idle windows that would re-throttle the HAM clock gate
    (PE drops to 1.2 GHz whenever it goes idle that long).

Layout choices:
  - All matmul inputs bf16 (host-cast), f32 PSUM accumulation.
  - x is staged transposed (x^T: [d, s]) so q^T/k^T ([e_local, s]) come
    straight out of the PE and serve as lhsT/rhs of the scores matmul.
  - Scores are computed transposed: S^T[kpos, q], contraction d=64 per
    head, two heads packed in the 128-row PE array via row tiling
    (tile_position (0,0)/(64,0)) writing one 2-bank PSUM pair, so a single
    ScalarE ACTIVATE does exp for both heads (amortizes the 352-cycle
    ACTIVATE overhead).
  - Softmax without max-subtraction (scores are O(10), exp is safe in f32),
    exp output bf16.
  - Causality at tile granularity: kpos-tile i of q-chunk j is skipped when
    fully masked; multiplied by a precomputed 0/1 mask on the 4 diagonal
    tiles, restricted to the first (r+1)*128 columns that can be masked.
  - P @ V via V_aug = [V | 1]: lhsT = V_aug [kpos, 65]; row 64 of the
    accumulated [65, q] output is the softmax denominator.
  - attn^T = out[0:64] * (1/denominator broadcast), via fast DVE reciprocal
    (18-bit; the exact one runs at 1/8 rate) + gpsimd partition broadcast.
    Custom ops (reciprocal_approx_fast / partition_broadcast) only get
    base-partition-0 SBUF inputs — they misbehave otherwise on HW.
  - y = attn^T.T @ w_o^T slices, f32 out.
"""

import numpy as np
import ml_dtypes

B = 2
S = 2048
D = 1024
H = 16
DK = 64
NCORES = 8
EL = 256  # local channels per core (4 heads)
QW = 512  # q-chunk width (free dim of scores matmuls)
NJ = S // QW  # 4 q-chunks

_CACHE = {}


def _build():
    import concourse.bass as bass
    import concourse.mybir as mybir
    import concourse.tile as tile
    from concourse import bacc

    f32 = mybir.dt.float32
    bf16 = mybir.dt.bfloat16
    ts = bass.ts
    Exp = mybir.ActivationFunctionType.Exp

    nc = bacc.Bacc("TRN2", num_devices=NCORES)
    xT_d = nc.dram_tensor("xT", [D, S], bf16, kind="ExternalInput")
    wqT_d = nc.dram_tensor("wqT", [D, EL], bf16, kind="ExternalInput")
    wkT_d = nc.dram_tensor("wkT", [D, EL], bf16, kind="ExternalInput")
    wvT_d = nc.dram_tensor("wvT", [D, EL], bf16, kind="ExternalInput")
    woT_d = nc.dram_tensor("woT", [EL, D], bf16, kind="ExternalInput")
    y_d = nc.dram_tensor("y", [S, D], f32, kind="ExternalOutput")

    DT = D // 128  # 8 d-tiles
    ST = S // 128  # 16 s-tiles

    with tile.TileContext(nc) as tc:
        with (
            tc.tile_pool(name="big", bufs=1) as big,
            tc.tile_pool(name="work", bufs=3) as work,
            tc.tile_pool(name="psum", bufs=1, space="PSUM") as psum,
        ):
            xT = big.tile([128, DT, S], bf16)  # x^T (d, s)
            wqT = big.tile([128, DT, EL], bf16)
            wkT = big.tile([128, DT, EL], bf16)
            wvT = big.tile([128, DT, EL], bf16)
            woT = big.tile([128, EL // 128, D], bf16)
            qT = big.tile([128, 2, S], bf16)  # (e_local, s)
            kT = big.tile([128, 2, S], bf16)
            vA = big.tile([128, ST, 4, DK + 1], bf16)  # (s%128, s//128, h, dv+1)
            aT = big.tile([128, 2, S], bf16)  # attn^T (d_local, s)
            masks = big.tile([128, 4, QW], bf16)

            # ---- constants: diagonal masks + ones column of V_aug ----
            # mask[r][p, f] = 1.0 iff f >= p + 128*r  (kpos <= q within tile)
            nc.gpsimd.memset(masks[:], 1.0)
            for r in range(4):
                nc.gpsimd.affine_select(
                    out=masks[:, r, :],
                    in_=masks[:, r, :],
                    compare_op=mybir.AluOpType.is_ge,
                    fill=0.0,
                    base=-128 * r,
                    pattern=[[1, QW]],
                    channel_multiplier=-1,
                )
            nc.gpsimd.memset(vA[:, :, :, DK], 1.0)

            # ---- input DMAs (qkv weights first; x split per (d-tile, s-half)
            # so the first projection matmuls can start early) ----
            nc.sync.dma_start(
                wqT[:], wqT_d.ap().rearrange("(ko p) e -> p ko e", p=128)
            )
            nc.sync.dma_start(
                wkT[:], wkT_d.ap().rearrange("(ko p) e -> p ko e", p=128)
            )
            nc.sync.dma_start(
                wvT[:], wvT_d.ap().rearrange("(ko p) e -> p ko e", p=128)
            )
            xT_r = xT_d.ap().rearrange("(ko p) s -> p ko s", p=128)
            for kd in range(DT):
                for sh in range(2):
                    nc.sync.dma_start(
                        xT[:, kd, ts(sh, S // 2)], xT_r[:, kd, ts(sh, S // 2)]
                    )
            nc.sync.dma_start(
                woT[:], woT_d.ap().rearrange("(ko p) e -> p ko e", p=128)
            )

            def qk_proj(sj):
                for w_sb, outT in ((wqT, qT), (wkT, kT)):
                    for et in range(2):
                        ps = psum.tile([128, QW], f32, tag="mm", bufs=1)
                        for kd in range(DT):
                            nc.tensor.matmul(
                                ps[:],
                                w_sb[:, kd, ts(et, 128)],
                                xT[:, kd, ts(sj, QW)],
                                start=(kd == 0),
                                stop=(kd == DT - 1),
                            )
                        nc.vector.tensor_copy(outT[:, et, ts(sj, QW)], ps[:])

            def v_proj(sj):
                for st in range(4 * sj, 4 * sj + 4):
                    ps = psum.tile([128, EL], f32, tag="mm", bufs=1)
                    for kd in range(DT):
                        nc.tensor.matmul(
                            ps[:],
                            xT[:, kd, ts(st, 128)],
                            wvT[:, kd, :],
                            start=(kd == 0),
                            stop=(kd == DT - 1),
                        )
                    nc.vector.tensor_copy(
                        vA[:, st, :, 0:DK],
                        ps[:].rearrange("p (h e) -> p h e", h=4),
                    )

            def stage_d(jd):
                for st in range(4 * jd, 4 * jd + 4):
                    for eo in range(2):
                        ps = psum.tile([128, QW], f32, tag="mm", bufs=1)
                        for kd in range(EL // 128):
                            nc.tensor.matmul(
                                ps[:],
                                aT[:, kd, ts(st, 128)],
                                woT[:, kd, ts(eo, QW)],
                                start=(kd == 0),
                                stop=(kd == EL // 128 - 1),
                            )
                        yt = work.tile([128, QW], f32, tag="yout", bufs=3)
                        nc.vector.tensor_copy(yt[:], ps[:])
                        nc.sync.dma_start(
                            y_d.ap().rearrange("(so p) e -> p so e", p=128)[
                                :, st, ts(eo, QW)
                            ],
                            yt[:],
                        )

            def attn_chunk(j):
                ilast = 4 * j + 3
                for hp in range(2):  # head pair = e-tile of qT/kT
                    if hp == 1 and j > 0:
                        stage_d(j - 1)
                    oa0 = psum.tile([128, QW], f32, tag="oa", bufs=3)
                    oa1 = psum.tile([128, QW], f32, tag="oa", bufs=3)
                    for i in range(ilast + 1):
                        sc = psum.tile([128, 2 * QW], f32, tag="sc", bufs=2)
                        nc.tensor.matmul(
                            sc[:, 0:QW],
                            kT[0:64, hp, ts(i, 128)],
                            qT[0:64, hp, ts(j, QW)],
                            tile_position=(0, 0),
                        )
                        nc.tensor.matmul(
                            sc[:, QW : 2 * QW],
                            kT[64:128, hp, ts(i, 128)],
                            qT[64:128, hp, ts(j, QW)],
                            tile_position=(64, 0),
                        )
                        e01 = work.tile([128, 2 * QW], bf16, tag="exps", bufs=4)
                        nc.scalar.activation(e01[:], sc[:], Exp, scale=0.125)
                        if i >= 4 * j:  # diagonal tile: causal mask
                            # only cols f < (r+1)*128 can be masked
                            r = i - 4 * j
                            mw = (r + 1) * 128
                            nc.vector.tensor_mul(
                                e01[:, 0:mw], e01[:, 0:mw], masks[:, r, 0:mw]
                            )
                            nc.vector.tensor_mul(
                                e01[:, QW : QW + mw],
                                e01[:, QW : QW + mw],
                                masks[:, r, 0:mw],
                            )
                        nc.tensor.matmul(
                            oa0[0 : DK + 1, :],
                            vA[:, i, 2 * hp, :],
                            e01[:, 0:QW],
                            start=(i == 0),
                            stop=(i == ilast),
                        )
                        nc.tensor.matmul(
                            oa1[0 : DK + 1, :],
                            vA[:, i, 2 * hp + 1, :],
                            e01[:, QW : 2 * QW],
                            start=(i == 0),
                            stop=(i == ilast),
                        )
                    for hh, oa in ((0, oa0), (1, oa1)):
                        h = 2 * hp + hh
                        dn = work.tile([1, QW], f32, tag="dn", bufs=2)
                        nc.vector.tensor_copy(dn[:], oa[DK : DK + 1, :])
                        rc = work.tile([1, QW], f32, tag="rc", bufs=2)
                        nc.vector.reciprocal_approx_fast(out=rc[:], in_=dn[:])
                        bc = work.tile([64, QW], f32, tag="bc", bufs=2)
                        nc.gpsimd.partition_broadcast(bc[:], rc[:])
                        nc.vector.tensor_mul(
                            aT[(h % 2) * 64 : (h % 2) * 64 + 64, h // 2, ts(j, QW)],
                            oa[0:DK, :],
                            bc[:],
                        )

            # ---- the pipeline ----
            for sj in range(NJ):
                qk_proj(sj)
                v_proj(sj)
                attn_chunk(sj)
            stage_d(NJ - 1)

    nc.compile()
    return nc


def _get_nc():
    if "nc" not in _CACHE:
        _CACHE["nc"] = _build()
    return _CACHE["nc"]


def kernel(x, w_q, w_k, w_v, w_o, _trace=False, _trace_cores=None):
    from concourse.bass_utils import run_bass_kernel_spmd

    nc = _get_nc()
    bf = ml_dtypes.bfloat16
    in_maps = []
    for c in range(NCORES):
        b = c // 4
        g = c % 4
        ch = slice(g * EL, (g + 1) * EL)
        in_maps.append(
            {
                "xT": np.ascontiguousarray(x[b].T).astype(bf),
                "wqT": np.ascontiguousarray(w_q[ch, :].T).astype(bf),
                "wkT": np.ascontiguousarray(w_k[ch, :].T).astype(bf),
                "wvT": np.ascontiguousarray(w_v[ch, :].T).astype(bf),
                "woT": np.ascontiguousarray(w_o[:, ch].T).astype(bf),
            }
        )
    res = run_bass_kernel_spmd(
        nc,
        in_maps,
        core_ids=list(range(NCORES)),
        trace=_trace,
        trace_cores=_trace_cores,
    )
    _CACHE["last_results"] = res
    y = np.zeros((B, S, D), np.float32)
    for c in range(NCORES):
        y[c // 4] += res.results[c]["y"]
    return y
